# revision 6
# baseline (speedup 1.0000x reference)
"""Trainium2 Bass kernel for batched general-score attention.

Reference computation (B=32, L=2048, H=2048):
    proj     = enc @ W^T + b          # [B, L, H]
    energies = proj . hidden          # [B, L]
    attn     = softmax(energies, 1)   # [B, L, 1]

Algebraic rewrite used here:
    energies = enc @ (W^T hidden) + (b . hidden)
The (b . hidden) term is constant across L for a batch, and softmax is
invariant to per-row constants, so it drops out entirely.  This collapses
the O(B*L*H^2) matmul into an O(B*H^2) matvec + O(B*L*H) batched dot.
The tiny matvec V = hidden @ W (134 MFLOP, 0.05% of the reference FLOPs)
is done host-side in fp32 BLAS while sharding the inputs.

fp16 + TensorEngine streaming: enc is transposed host-side to [H, L] per
batch and downcast to fp16 (halves HBM traffic: 32 MB/core).  The batched
dot runs on the PE array as a matvec with the u-vector chunks as
stationary weights:

    e[l] = sum_k  u[k*128:(k+1)*128]^T @ encT[k*128:(k+1)*128, l]

i.e. per batch 16 h-chunks x 4 L-chunks of matmul([128,1]^T @ [128,512])
accumulating into four [1,512] PSUM banks (start at k=0, stop at k=15).
The DVE scalar_tensor_tensor path used in an earlier revision has no
fast perf mode (1x only -> 146 us/core); the tensor engine is ~2.5x
faster and leaves DVE free for softmax.

PE pstate filler: the PE clock ramps 1.2 -> 2.4 GHz only under sustained
load, and drops back during DMA-wait stalls (measured: 427 ns vs 216 ns
per [128,1]x[128,512] matmul).  Since full-speed PE outruns the ~400
GB/s DMA stream, it stalls every chunk and oscillates between pstates,
trailing the stream by ~10-20 us.  Dummy matmuls (never-read PSUM bank)
pad each chunk's real work up to the DMA pace so the PE stays busy and
holds the fast pstate.

Softmax per batch on the [1, 2048] energy row (partition 0 only):
per-L-chunk PSUM->SBUF copies (ACT) pipelined with per-chunk partial
maxes (DVE), ACT exp (bias = -max) with accumulated sum, DVE reciprocal
+ scale, then one contiguous 8 KB DMA into the output row.

Sharding: data-parallel over batch.  8 cores x 4 batches each.
Accuracy (vs fp32 reference, measured on the real seed-0 data): rel err
~6e-3 against a 2e-2 gate; energies have top-2 gaps >> the fp16-induced
~0.06 energy noise.
"""

import sys

if "/opt/trn_rl_repo" not in sys.path:
    sys.path.insert(0, "/opt/trn_rl_repo")

from contextlib import ExitStack

import numpy as np

import concourse.bacc as bacc
import concourse.bass as bass
import concourse.mybir as mybir
import concourse.tile as tile
from concourse._compat import with_exitstack
from concourse.bass_utils import run_bass_kernel_spmd

B, L, H = 32, 2048, 2048
N_CORES = 8
BL = B // N_CORES  # batches per core
P = 128            # partitions
HK = H // P        # h-chunks per batch (16)
NJ = 4             # L-chunks of 512 per batch
LJ = L // NJ       # 512

F16 = mybir.dt.float16
F32 = mybir.dt.float32

FILL_START = 14    # dummy matmuls before the first chunk's work
FILL_CHUNK = 7     # dummy matmuls ahead of each later chunk's work


@with_exitstack
def _attn_kernel(ctx: ExitStack, tc: tile.TileContext,
                 enc: bass.AP, v: bass.AP, out: bass.AP):
    nc = tc.nc

    singles = ctx.enter_context(tc.tile_pool(name="singles", bufs=1))
    encpool = ctx.enter_context(tc.tile_pool(name="encpool", bufs=8))
    small = ctx.enter_context(tc.tile_pool(name="small", bufs=2))
    psum = ctx.enter_context(tc.tile_pool(name="psum", bufs=7, space="PSUM"))
    dpsum = ctx.enter_context(tc.tile_pool(name="dpsum", bufs=1, space="PSUM"))

    # ---- issue the gating DMAs first: u vectors (16 KB, host-packed as
    # [128, BL*HK] with column (b*HK+k) = u_b[k*128:(k+1)*128]) on the
    # ScalarE ring, first enc chunk right behind on the SyncE ring (the
    # main loop below emits it).
    v_sb = singles.tile([P, BL * HK], F16)
    nc.scalar.dma_start(out=v_sb, in_=v)

    # PE pstate filler: matmuls into a never-read PSUM bank.
    dummy_rhs = singles.tile([P, LJ], F16)
    nc.vector.memset(dummy_rhs, 0.0)
    dummy_ps = dpsum.tile([1, LJ], F32)

    def pe_fill(n):
        for _ in range(n):
            nc.tensor.matmul(dummy_ps, lhsT=v_sb[:, 0:1], rhs=dummy_rhs,
                             start=True, stop=True)

    chunk_idx = 0
    for b in range(BL):
        e_ps = [psum.tile([1, LJ], F32, tag="eps", name=f"eps{j}")
                for j in range(NJ)]
        e_b = small.tile([1, L], F32, tag="e")
        m4 = small.tile([1, NJ], F32, tag="m4")
        if b == 0:
            # small chunks first so the PE starts sooner
            plan = [(0, 1), (1, 1), (2, 2), (4, 4), (8, 4), (12, 4)]
        elif b == BL - 1:
            # descending sizes at the end shorten the last-arrival tail
            plan = [(0, 4), (4, 4), (8, 4), (12, 2), (14, 1), (15, 1)]
        else:
            plan = [(4 * i, 4) for i in range(4)]
        for t_start, ntile in plan:
            enc_t = encpool.tile([P, 4, L], F16)
            row0 = (b * HK + t_start) * P
            # alternate the two HWDGE rings so more transfers are in
            # flight and one ring's completion hiccup doesn't stall
            ring = nc.sync if chunk_idx % 2 == 0 else nc.scalar
            ring.dma_start(
                out=enc_t[:, 0:ntile, :],
                in_=enc[row0:row0 + ntile * P, :].rearrange(
                    "(n p) l -> p n l", p=P))
            # keep the PE busy across the DMA wait so it holds the fast
            # pstate (skip after the final chunk: nothing left to hide)
            pe_fill(FILL_START if chunk_idx == 0 else FILL_CHUNK)
            chunk_idx += 1
            for i in range(ntile):
                k = t_start + i
                u_bk = v_sb[:, b * HK + k:b * HK + k + 1]
                for j in range(NJ):
                    nc.tensor.matmul(
                        e_ps[j],
                        lhsT=u_bk,
                        rhs=enc_t[:, i, j * LJ:(j + 1) * LJ],
                        start=(k == 0), stop=(k == HK - 1))
        # drain PSUM -> SBUF energy row (ACT), partial maxes per chunk
        # (DVE) pipelined behind the copies
        for j in range(NJ):
            nc.scalar.copy(e_b[:, j * LJ:(j + 1) * LJ], e_ps[j])
            nc.vector.reduce_max(m4[:, j:j + 1], e_b[:, j * LJ:(j + 1) * LJ],
                                 axis=mybir.AxisListType.X)
        m = small.tile([1, 1], F32, tag="m")
        nc.vector.reduce_max(m, m4, axis=mybir.AxisListType.X)
        neg_m = small.tile([1, 1], F32, tag="negm")
        nc.vector.tensor_scalar_mul(neg_m, m, -1.0)
        p_un = small.tile([1, L], F32, tag="p")
        s = small.tile([1, 1], F32, tag="s")
        nc.scalar.activation(
            p_un, e_b, mybir.ActivationFunctionType.Exp,
            bias=neg_m[0:1, 0:1], accum_out=s)
        r = small.tile([1, 1], F32, tag="r")
        nc.vector.reciprocal(r, s)
        attn = small.tile([1, L], F32, tag="attn")
        nc.vector.tensor_scalar_mul(attn, p_un, r[0:1, 0:1])
        # contiguous 8 KB row store; ScalarE ring so the SyncE enc stream
        # never head-of-line blocks on it.
        nc.scalar.dma_start(out=out[b:b + 1, :], in_=attn)


def build_program():
    nc = bacc.Bacc("TRN2", target_bir_lowering=False, debug=False,
                   enable_asserts=False, num_devices=N_CORES)
    enc = nc.dram_tensor("enc", [BL * H, L], F16, kind="ExternalInput")
    v = nc.dram_tensor("v", [P, BL * HK], F16, kind="ExternalInput")
    out = nc.dram_tensor("out", [BL, L], F32, kind="ExternalOutput")
    with tile.TileContext(nc) as tc:
        _attn_kernel(tc, enc.ap(), v.ap(), out.ap())
    nc.compile()
    return nc


_NC_CACHE = {}


def _get_program():
    if "nc" not in _NC_CACHE:
        _NC_CACHE["nc"] = build_program()
    return _NC_CACHE["nc"]


def make_in_maps(hidden, encoder_outputs, W):
    hidden = np.asarray(hidden, dtype=np.float32)
    encoder_outputs = np.asarray(encoder_outputs)
    W = np.asarray(W, dtype=np.float32)
    V = (hidden[:, 0, :] @ W).astype(np.float16)  # [B, H]
    enc16 = encoder_outputs.astype(np.float16)
    in_maps = []
    for c in range(N_CORES):
        b0 = c * BL
        # [BL, L, H] -> [BL, H, L] transposed, contiguous
        encT = np.ascontiguousarray(
            enc16[b0:b0 + BL].transpose(0, 2, 1)).reshape(BL * H, L)
        # u pack: [128, BL*HK], column (b*HK+k) = V[b0+b, k*128:(k+1)*128]
        vpack = np.ascontiguousarray(
            V[b0:b0 + BL].reshape(BL, HK, P).transpose(2, 0, 1)
        ).reshape(P, BL * HK)
        in_maps.append({"enc": encT, "v": vpack})
    return in_maps


def kernel(hidden, encoder_outputs, W, b, **_):
    nc = _get_program()
    in_maps = make_in_maps(hidden, encoder_outputs, W)
    res = run_bass_kernel_spmd(nc, in_maps, core_ids=list(range(N_CORES)))
    out = np.concatenate(
        [res.results[c]["out"].reshape(BL, L, 1) for c in range(N_CORES)],
        axis=0)
    return out.astype(np.float32)


# revision 7
# speedup vs baseline: 1.0276x; 1.0276x over previous
"""Trainium2 Bass kernel for batched general-score attention.

Reference computation (B=32, L=2048, H=2048):
    proj     = enc @ W^T + b          # [B, L, H]
    energies = proj . hidden          # [B, L]
    attn     = softmax(energies, 1)   # [B, L, 1]

Algebraic rewrite used here:
    energies = enc @ (W^T hidden) + (b . hidden)
The (b . hidden) term is constant across L for a batch, and softmax is
invariant to per-row constants, so it drops out entirely.  This collapses
the O(B*L*H^2) matmul into an O(B*H^2) matvec + O(B*L*H) batched dot.
The tiny matvec V = hidden @ W (134 MFLOP, 0.05% of the reference FLOPs)
is done host-side in fp32 BLAS while sharding the inputs.

Work split (all fp16 uploads, 34 MB/core, DMA-bound at ~400 GB/s):
 *  L-rows 0..1535 of each batch go to the TensorEngine: host-transposed
    to [H, 1536], streamed as [128, n, 1536] tiles; per batch 16 h-chunks
    x 3 L-chunks of matmul([128,1]^T @ [128,512]) with the u-vector
    chunks as stationary weights, accumulating into three [1,512] PSUM
    banks (start k=0, stop k=15).
 *  L-rows 1536..2047 stay in natural [512, H] layout and go to the DVE:
    four [128, 2048] scalar_tensor_tensor tiles (multiply by the
    pre-broadcast u row + fused row-sum) -> energies [128, 4].
The split keeps the PE under the DMA pace even at its slow pstate (the
PE clock ramps 1.2 -> 2.4 GHz only under sustained load and drops back
during DMA stalls; at 1.2 GHz the PE alone would trail the stream by
~10-20 us/core).  DVE is otherwise idle and absorbs 25% of the dot work.

The DVE-part energies [128, 4] are folded back onto partition 0 via
PE-transpose -> [4, 128] -> ACT copy -> 2 KB DRAM bounce -> e_b[0,
1536:2048], well off the critical path, so softmax stays a single
unified [1, 2048] row per batch: per-chunk partial maxes (DVE), ACT exp
(bias=-max) with accumulated sum, DVE reciprocal + scale, one contiguous
8 KB output DMA.

Sharding: data-parallel over batch.  8 cores x 4 batches each.
Accuracy (vs fp32 reference, measured on the real seed-0 data): rel err
~6e-3 against a 2e-2 gate.
"""

import sys

if "/opt/trn_rl_repo" not in sys.path:
    sys.path.insert(0, "/opt/trn_rl_repo")

from contextlib import ExitStack

import numpy as np

import concourse.bacc as bacc
import concourse.bass as bass
import concourse.mybir as mybir
import concourse.tile as tile
from concourse._compat import with_exitstack
from concourse.bass_utils import run_bass_kernel_spmd

B, L, H = 32, 2048, 2048
N_CORES = 8
BL = B // N_CORES   # batches per core
P = 128             # partitions
HK = H // P         # h-chunks per batch (16)
NJ = 3              # PE L-chunks of 512 per batch (L 0..1535)
LJ = 512
LPE = NJ * LJ       # 1536, PE part
LDV = L - LPE       # 512, DVE part
ND = LDV // P       # 4 natural sub-tiles

F16 = mybir.dt.float16
F32 = mybir.dt.float32


@with_exitstack
def _attn_kernel(ctx: ExitStack, tc: tile.TileContext, enc: bass.AP,
                 encn: bass.AP, v: bass.AP, vb_d: bass.AP, scr: bass.AP,
                 out: bass.AP):
    nc = tc.nc

    singles = ctx.enter_context(tc.tile_pool(name="singles", bufs=1))
    encpool = ctx.enter_context(tc.tile_pool(name="encpool", bufs=6))
    natpool = ctx.enter_context(tc.tile_pool(name="natpool", bufs=2))
    vbpool = ctx.enter_context(tc.tile_pool(name="vbpool", bufs=BL))
    scratch = ctx.enter_context(tc.tile_pool(name="scratch", bufs=2))
    small = ctx.enter_context(tc.tile_pool(name="small", bufs=2))
    psum = ctx.enter_context(tc.tile_pool(name="psum", bufs=6, space="PSUM"))
    ptpool = ctx.enter_context(tc.tile_pool(name="ptpool", bufs=2,
                                            space="PSUM"))

    # u chunks for the PE (16 KB) and u row broadcast for vb[0] — both on
    # the ScalarE ring, ahead of everything else it carries.
    v_sb = singles.tile([P, BL * HK], F16)
    nc.scalar.dma_start(out=v_sb, in_=v)
    vb = []
    for _i in range(BL):
        vb_b = vbpool.tile([P, H], F16, tag="vb_b")
        vb.append(vb_b)
    nc.scalar.dma_start(out=vb[0], in_=vb_d[0:P, :])

    # identity for the PE-transpose of the DVE-part energies
    ident_dram = nc.inline_tensor(np.eye(P, dtype=np.float32), name="ident")
    ident = singles.tile([P, P], F32)
    nc.scalar.dma_start(out=ident, in_=ident_dram.ap())

    chunk_idx = 0
    for b in range(BL):
        e_ps = [psum.tile([1, LJ], F32, tag="eps", name=f"eps{j}")
                for j in range(NJ)]
        e_b = small.tile([1, L], F32, tag="e")
        eB = small.tile([P, ND], F32, tag="eB")
        m4 = small.tile([1, NJ + 1], F32, tag="m4")

        if b == 0:
            # two small PE chunks first so the PE starts sooner; the
            # natural (DVE) chunk follows them — DVE has slack.
            plan = [(0, 1), (1, 1), ("nat", 0), (2, 2), (4, 4), (8, 4),
                    (12, 4)]
        elif b == BL - 1:
            plan = [("nat", 0), (0, 4), (4, 4), (8, 4), (12, 2), (14, 1),
                    (15, 1)]
        else:
            plan = [("nat", 0)] + [(4 * i, 4) for i in range(4)]

        pe_chunks_done = 0
        n_pe_chunks = sum(1 for t, _ in plan if t != "nat")
        for t_start, ntile in plan:
            ring = nc.sync if chunk_idx % 2 == 0 else nc.scalar
            chunk_idx += 1
            if t_start == "nat":
                nat_t = natpool.tile([P, ND, H], F16)
                r0 = b * LDV
                ring.dma_start(
                    out=nat_t,
                    in_=encn[r0:r0 + LDV, :].rearrange(
                        "(n p) h -> p n h", p=P))
                for i in range(ND):
                    prod = scratch.tile([P, H], F16)
                    nc.vector.scalar_tensor_tensor(
                        out=prod, in0=nat_t[:, i, :], scalar=1.0,
                        in1=vb[b], op0=mybir.AluOpType.mult,
                        op1=mybir.AluOpType.mult,
                        accum_out=eB[:, i:i + 1])
                continue
            enc_t = encpool.tile([P, 4, LPE], F16)
            row0 = (b * HK + t_start) * P
            ring.dma_start(
                out=enc_t[:, 0:ntile, :],
                in_=enc[row0:row0 + ntile * P, :].rearrange(
                    "(n p) l -> p n l", p=P))
            # deferred vb loads, well ahead of their use at b=1..3
            if b == 0 and t_start in (2, 4, 8):
                vbi = {2: 1, 4: 2, 8: 3}[t_start]
                nc.scalar.dma_start(out=vb[vbi],
                                    in_=vb_d[vbi * P:(vbi + 1) * P, :])
            for i in range(ntile):
                k = t_start + i
                u_bk = v_sb[:, b * HK + k:b * HK + k + 1]
                for j in range(NJ):
                    nc.tensor.matmul(
                        e_ps[j],
                        lhsT=u_bk,
                        rhs=enc_t[:, i, j * LJ:(j + 1) * LJ],
                        start=(k == 0), stop=(k == HK - 1))
            pe_chunks_done += 1
            if pe_chunks_done == n_pe_chunks - 1:
                # fold the DVE energies onto partition 0: PE transpose,
                # ACT copy, 2 KB DRAM bounce into e_b[0, 1536:2048].
                # Emitted second-to-last so the PE never waits on eB.
                pt = ptpool.tile([ND, P], F32, tag="pt")
                nc.tensor.transpose(pt, eB, ident)
                sb4 = small.tile([ND, P], F32, tag="sb4")
                nc.scalar.copy(sb4, pt)
                nc.scalar.dma_start(out=scr[b:b + 1, :], in_=sb4)
                nc.scalar.dma_start(out=e_b[0:1, LPE:L], in_=scr[b:b + 1, :])

        # drain PSUM -> SBUF energy row (ACT), partial maxes per chunk
        # (DVE) pipelined behind the copies
        for j in range(NJ):
            nc.scalar.copy(e_b[:, j * LJ:(j + 1) * LJ], e_ps[j])
            nc.vector.reduce_max(m4[:, j:j + 1], e_b[:, j * LJ:(j + 1) * LJ],
                                 axis=mybir.AxisListType.X)
        nc.vector.reduce_max(m4[:, NJ:NJ + 1], e_b[:, LPE:L],
                             axis=mybir.AxisListType.X)
        m = small.tile([1, 1], F32, tag="m")
        nc.vector.reduce_max(m, m4, axis=mybir.AxisListType.X)
        neg_m = small.tile([1, 1], F32, tag="negm")
        nc.vector.tensor_scalar_mul(neg_m, m, -1.0)
        p_un = small.tile([1, L], F32, tag="p")
        s = small.tile([1, 1], F32, tag="s")
        nc.scalar.activation(
            p_un, e_b, mybir.ActivationFunctionType.Exp,
            bias=neg_m[0:1, 0:1], accum_out=s)
        r = small.tile([1, 1], F32, tag="r")
        nc.vector.reciprocal(r, s)
        attn = small.tile([1, L], F32, tag="attn")
        nc.vector.tensor_scalar_mul(attn, p_un, r[0:1, 0:1])
        # contiguous 8 KB row store; ScalarE ring so the SyncE enc stream
        # never head-of-line blocks on it.
        nc.scalar.dma_start(out=out[b:b + 1, :], in_=attn)


def build_program():
    nc = bacc.Bacc("TRN2", target_bir_lowering=False, debug=False,
                   enable_asserts=False, num_devices=N_CORES)
    enc = nc.dram_tensor("enc", [BL * H, LPE], F16, kind="ExternalInput")
    encn = nc.dram_tensor("encn", [BL * LDV, H], F16, kind="ExternalInput")
    v = nc.dram_tensor("v", [P, BL * HK], F16, kind="ExternalInput")
    vb_d = nc.dram_tensor("vb", [BL * P, H], F16, kind="ExternalInput")
    scr = nc.dram_tensor("scr", [BL, LDV], F32, kind="Internal")
    out = nc.dram_tensor("out", [BL, L], F32, kind="ExternalOutput")
    with tile.TileContext(nc) as tc:
        _attn_kernel(tc, enc.ap(), encn.ap(), v.ap(), vb_d.ap(), scr.ap(),
                     out.ap())
    nc.compile()
    return nc


_NC_CACHE = {}


def _get_program():
    if "nc" not in _NC_CACHE:
        _NC_CACHE["nc"] = build_program()
    return _NC_CACHE["nc"]


def make_in_maps(hidden, encoder_outputs, W):
    hidden = np.asarray(hidden, dtype=np.float32)
    encoder_outputs = np.asarray(encoder_outputs)
    W = np.asarray(W, dtype=np.float32)
    V = (hidden[:, 0, :] @ W).astype(np.float16)  # [B, H]
    enc16 = encoder_outputs.astype(np.float16)
    Vb = np.ascontiguousarray(
        np.broadcast_to(V[:, None, :], (B, P, H)))  # [B, 128, H] fp16
    in_maps = []
    for c in range(N_CORES):
        b0 = c * BL
        # PE part: [BL, H, LPE] transposed, contiguous
        encT = np.ascontiguousarray(
            enc16[b0:b0 + BL, :LPE, :].transpose(0, 2, 1)).reshape(
                BL * H, LPE)
        # DVE part: natural layout [BL*LDV, H]
        encN = np.ascontiguousarray(
            enc16[b0:b0 + BL, LPE:, :]).reshape(BL * LDV, H)
        # u pack: [128, BL*HK], column (b*HK+k) = V[b0+b, k*128:(k+1)*128]
        vpack = np.ascontiguousarray(
            V[b0:b0 + BL].reshape(BL, HK, P).transpose(2, 0, 1)
        ).reshape(P, BL * HK)
        in_maps.append({
            "enc": encT, "encn": encN, "v": vpack,
            "vb": Vb[b0:b0 + BL].reshape(BL * P, H),
        })
    return in_maps


def kernel(hidden, encoder_outputs, W, b, **_):
    nc = _get_program()
    in_maps = make_in_maps(hidden, encoder_outputs, W)
    res = run_bass_kernel_spmd(nc, in_maps, core_ids=list(range(N_CORES)))
    out = np.concatenate(
        [res.results[c]["out"].reshape(BL, L, 1) for c in range(N_CORES)],
        axis=0)
    return out.astype(np.float32)


# revision 9
# speedup vs baseline: 1.1872x; 1.1553x over previous
"""Trainium2 Bass kernel for batched general-score attention.

Reference computation (B=32, L=2048, H=2048):
    proj     = enc @ W^T + b          # [B, L, H]
    energies = proj . hidden          # [B, L]
    attn     = softmax(energies, 1)   # [B, L, 1]

Algebraic rewrite used here:
    energies = enc @ (W^T hidden) + (b . hidden)
The (b . hidden) term is constant across L for a batch, and softmax is
invariant to per-row constants, so it drops out entirely.  This collapses
the O(B*L*H^2) matmul into an O(B*H^2) matvec + O(B*L*H) batched dot.
The tiny matvec V = hidden @ W (134 MFLOP, 0.05% of the reference FLOPs)
is done host-side in fp32 BLAS while sharding the inputs.

fp16 + TensorEngine streaming: enc is transposed host-side to [H, L] per
batch and downcast to fp16 (halves HBM traffic: 32 MB/core, and the DMA
sustains ~400 GB/s/core with [128, 4, L] tiles).  The batched dot runs
on the PE array as a matvec with the u-vector chunks as stationary
weights:

    e[l] = sum_k  u[k*128:(k+1)*128]^T @ encT[k*128:(k+1)*128, l]

i.e. per batch 16 h-chunks x 4 L-chunks of matmul([128,1]^T @ [128,512])
accumulating into four [1,512] PSUM banks (start at k=0, stop at k=15).
The DVE scalar_tensor_tensor alternative has no fast perf mode (1x only
-> 146 us/core); the tensor engine path is ~2.5x faster.

Engine-stream hygiene (HWDGE DMAs execute in the issuing engine's
in-order stream): the output DMAs go on the GpSimd SWDGE ring, which is
otherwise idle, so the ACT/Sync rings never head-of-line block on a
softmax dependency.  PSUM->SBUF drains alternate ACT/DVE, each followed
by its per-chunk partial max on DVE, so the reduce of the full row never
appears on the critical tail.

Softmax per batch on the [1, 2048] energy row (partition 0 only):
partial chunk maxes -> [1,4] -> max, ACT exp (bias=-max) with
accumulated sum, DVE reciprocal + scale, one contiguous 8 KB output DMA.

Sharding: data-parallel over batch.  8 cores x 4 batches each.
Accuracy (vs fp32 reference, measured on the real seed-0 data): rel err
~6e-3 against a 2e-2 gate.
"""

import sys

if "/opt/trn_rl_repo" not in sys.path:
    sys.path.insert(0, "/opt/trn_rl_repo")

from contextlib import ExitStack

import numpy as np

import concourse.bacc as bacc
import concourse.bass as bass
import concourse.mybir as mybir
import concourse.tile as tile
from concourse._compat import with_exitstack
from concourse.bass_utils import run_bass_kernel_spmd

B, L, H = 32, 2048, 2048
N_CORES = 8
BL = B // N_CORES  # batches per core
P = 128            # partitions
HK = H // P        # h-chunks per batch (16)
NJ = 4             # L-chunks of 512 per batch
LJ = L // NJ       # 512

F16 = mybir.dt.float16
F32 = mybir.dt.float32


@with_exitstack
def _attn_kernel(ctx: ExitStack, tc: tile.TileContext,
                 enc: bass.AP, v: bass.AP, out: bass.AP):
    nc = tc.nc

    singles = ctx.enter_context(tc.tile_pool(name="singles", bufs=1))
    encpool = ctx.enter_context(tc.tile_pool(name="encpool", bufs=8))
    small = ctx.enter_context(tc.tile_pool(name="small", bufs=2))
    psum = ctx.enter_context(tc.tile_pool(name="psum", bufs=8, space="PSUM"))

    # u vectors: one 16 KB DMA, host-packed as [128, BL*HK] where column
    # (b*HK + k) holds u_b[k*128 : (k+1)*128].  First on the ScalarE ring.
    v_sb = singles.tile([P, BL * HK], F16)
    nc.scalar.dma_start(out=v_sb, in_=v)

    chunk_idx = 0
    for b in range(BL):
        e_ps = [psum.tile([1, LJ], F32, tag="eps", name=f"eps{j}")
                for j in range(NJ)]
        e_b = small.tile([1, L], F32, tag="e")
        m4 = small.tile([1, NJ], F32, tag="m4")
        if b == 0:
            # small chunks first so the PE starts sooner
            plan = [(0, 1), (1, 1), (2, 2), (4, 4), (8, 4), (12, 4)]
        elif b == BL - 1:
            # descending sizes at the end shorten the last-arrival tail
            plan = [(0, 4), (4, 4), (8, 4), (12, 2), (14, 1), (15, 1)]
        else:
            plan = [(4 * i, 4) for i in range(4)]
        for t_start, ntile in plan:
            enc_t = encpool.tile([P, 4, L], F16)
            row0 = (b * HK + t_start) * P
            # alternate the two HWDGE rings so more transfers are in
            # flight and one ring's completion hiccup doesn't stall
            ring = nc.sync if chunk_idx % 2 == 0 else nc.scalar
            chunk_idx += 1
            ring.dma_start(
                out=enc_t[:, 0:ntile, :],
                in_=enc[row0:row0 + ntile * P, :].rearrange(
                    "(n p) l -> p n l", p=P))
            for i in range(ntile):
                k = t_start + i
                u_bk = v_sb[:, b * HK + k:b * HK + k + 1]
                for j in range(NJ):
                    nc.tensor.matmul(
                        e_ps[j],
                        lhsT=u_bk,
                        rhs=enc_t[:, i, j * LJ:(j + 1) * LJ],
                        start=(k == 0), stop=(k == HK - 1))
        # drain PSUM -> SBUF energy row, alternating ACT/DVE so the two
        # copies pipeline; each chunk's partial max follows on DVE
        for j in range(NJ):
            if j % 2 == 0:
                nc.scalar.copy(e_b[:, j * LJ:(j + 1) * LJ], e_ps[j])
            else:
                nc.vector.tensor_copy(e_b[:, j * LJ:(j + 1) * LJ], e_ps[j])
            nc.vector.reduce_max(m4[:, j:j + 1], e_b[:, j * LJ:(j + 1) * LJ],
                                 axis=mybir.AxisListType.X)
        m = small.tile([1, 1], F32, tag="m")
        nc.vector.reduce_max(m, m4, axis=mybir.AxisListType.X)
        neg_m = small.tile([1, 1], F32, tag="negm")
        nc.vector.tensor_scalar_mul(neg_m, m, -1.0)
        p_un = small.tile([1, L], F32, tag="p")
        s = small.tile([1, 1], F32, tag="s")
        nc.scalar.activation(
            p_un, e_b, mybir.ActivationFunctionType.Exp,
            bias=neg_m[0:1, 0:1], accum_out=s)
        r = small.tile([1, 1], F32, tag="r")
        nc.vector.reciprocal(r, s)
        attn = small.tile([1, L], F32, tag="attn")
        nc.vector.tensor_scalar_mul(attn, p_un, r[0:1, 0:1])
        # contiguous 8 KB row store on the idle GpSimd SWDGE ring: its
        # softmax dependency never blocks the HWDGE enc streams.
        nc.gpsimd.dma_start(out=out[b:b + 1, :], in_=attn)


def build_program():
    nc = bacc.Bacc("TRN2", target_bir_lowering=False, debug=False,
                   enable_asserts=False, num_devices=N_CORES)
    enc = nc.dram_tensor("enc", [BL * H, L], F16, kind="ExternalInput")
    v = nc.dram_tensor("v", [P, BL * HK], F16, kind="ExternalInput")
    out = nc.dram_tensor("out", [BL, L], F32, kind="ExternalOutput")
    with tile.TileContext(nc) as tc:
        _attn_kernel(tc, enc.ap(), v.ap(), out.ap())
    nc.compile()
    return nc


_NC_CACHE = {}


def _get_program():
    if "nc" not in _NC_CACHE:
        _NC_CACHE["nc"] = build_program()
    return _NC_CACHE["nc"]


def make_in_maps(hidden, encoder_outputs, W):
    hidden = np.asarray(hidden, dtype=np.float32)
    encoder_outputs = np.asarray(encoder_outputs)
    W = np.asarray(W, dtype=np.float32)
    V = (hidden[:, 0, :] @ W).astype(np.float16)  # [B, H]
    enc16 = encoder_outputs.astype(np.float16)
    in_maps = []
    for c in range(N_CORES):
        b0 = c * BL
        # [BL, L, H] -> [BL, H, L] transposed, contiguous
        encT = np.ascontiguousarray(
            enc16[b0:b0 + BL].transpose(0, 2, 1)).reshape(BL * H, L)
        # u pack: [128, BL*HK], column (b*HK+k) = V[b0+b, k*128:(k+1)*128]
        vpack = np.ascontiguousarray(
            V[b0:b0 + BL].reshape(BL, HK, P).transpose(2, 0, 1)
        ).reshape(P, BL * HK)
        in_maps.append({"enc": encT, "v": vpack})
    return in_maps


def kernel(hidden, encoder_outputs, W, b, **_):
    nc = _get_program()
    in_maps = make_in_maps(hidden, encoder_outputs, W)
    res = run_bass_kernel_spmd(nc, in_maps, core_ids=list(range(N_CORES)))
    out = np.concatenate(
        [res.results[c]["out"].reshape(BL, L, 1) for c in range(N_CORES)],
        axis=0)
    return out.astype(np.float32)
